# revision 4
# baseline (speedup 1.0000x reference)
"""Top-k (k=3) row masking + renormalize, data-parallel across 8 NeuronCores.

Input  x: [128, 512, 512] f32. For each row (last axis): keep the top-3
entries (counting duplicates), zero the rest, scale kept entries by the
reciprocal of their sum.

Per-core algorithm (rows are independent; batch dim sharded 8 ways):
  - vector.max   -> top-8 values per row; entry [2] == 3rd largest == kth
  - scalar_tensor_tensor: v = (x >= kth) * x, fused row-sum accum -> s
  - vector.reciprocal: inv = 1/s  (batched over CHUNK row-blocks)
  - scalar.mul (ACT engine): out = v * inv
This is exactly the reference computation (same mask semantics incl. ties).

Layout: rows are assigned partition-major ("(p n) d"), so each partition's
blocks are contiguous in HBM -> each chunk DMA moves CHUNK*2KB contiguous
bytes per partition (large descriptors, near-peak HBM bandwidth).
"""

import sys

import numpy as np

if "/opt/trn_rl_repo" not in sys.path:
    sys.path.insert(0, "/opt/trn_rl_repo")

N_CORES = 8
B, L1, D = 128, 512, 512
ROWS_PER_CORE = (B // N_CORES) * L1  # 8192
NBLK = ROWS_PER_CORE // 128  # 64 blocks of [128, 512]
# Blocks per DMA chunk: small chunks at the ends shorten pipeline ramp
# (first compute can start after a 512KB transfer instead of 2MB) and
# drain (tail flush is recip + 2 ACT muls + a 512KB store).
CHUNKS = [2, 2, 4, 8, 8, 8, 8, 8, 8, 4, 2, 2]
assert sum(CHUNKS) == NBLK

_PROGRAM = None


def _build_program():
    from concourse import bacc, bass, tile

    mybir = bass.mybir
    f32 = mybir.dt.float32

    # Bacc (not raw Bass): its compile pass legalizes Tile's multi-wait
    # instructions, which walrus codegen rejects (one wait slot per inst).
    nc = bacc.Bacc("TRN2", target_bir_lowering=False, debug=False)
    x_in = nc.dram_tensor("x", [ROWS_PER_CORE, D], f32, kind="ExternalInput")
    y_out = nc.dram_tensor("y", [ROWS_PER_CORE, D], f32, kind="ExternalOutput")

    # [8192, 512] -> [128 partitions, 64 blocks, 512]; row (p*64+n) -> [p, n, :]
    # Partition-major: each partition's 64 blocks are contiguous in HBM.
    xv = x_in[:].rearrange("(p n) d -> p n d", n=NBLK)
    yv = y_out[:].rearrange("(p n) d -> p n d", n=NBLK)

    with tile.TileContext(nc) as tc:
        with (
            tc.tile_pool(name="xp", bufs=4) as xp,
            tc.tile_pool(name="vp", bufs=2) as vp,
            tc.tile_pool(name="op", bufs=2) as op_pool,
            tc.tile_pool(name="small", bufs=3) as sp,
        ):
            base = 0
            for chunk in CHUNKS:
                sl = slice(base, base + chunk)
                base += chunk
                xt = xp.tile([128, chunk, D], f32, tag="xt")
                vt = vp.tile([128, chunk, D], f32, tag="vt")
                ot = op_pool.tile([128, chunk, D], f32, tag="ot")
                t8 = sp.tile([128, chunk, 8], f32, tag="t8")
                st = sp.tile([128, chunk], f32, tag="st")
                iv = sp.tile([128, chunk], f32, tag="iv")

                nc.sync.dma_start(out=xt[:], in_=xv[:, sl, :])

                for j in range(chunk):
                    nc.vector.max(out=t8[:, j, :], in_=xt[:, j, :])
                    nc.vector.scalar_tensor_tensor(
                        out=vt[:, j, :],
                        in0=xt[:, j, :],
                        scalar=t8[:, j, 2:3],
                        in1=xt[:, j, :],
                        op0=mybir.AluOpType.is_ge,
                        op1=mybir.AluOpType.mult,
                        accum_out=st[:, j : j + 1],
                    )
                nc.vector.reciprocal(out=iv[:], in_=st[:])
                for j in range(chunk):
                    nc.scalar.mul(
                        out=ot[:, j, :], in_=vt[:, j, :], mul=iv[:, j : j + 1]
                    )

                nc.scalar.dma_start(out=yv[:, sl, :], in_=ot[:])

    nc.finalize()
    return nc


def _get_program():
    global _PROGRAM
    if _PROGRAM is None:
        _PROGRAM = _build_program()
    return _PROGRAM


def kernel(x: np.ndarray, _trace: bool = False):
    from concourse.bass_utils import run_bass_kernel_spmd

    x = np.ascontiguousarray(x, dtype=np.float32)
    assert x.shape == (B, L1, D), x.shape
    per = B // N_CORES
    in_maps = [
        {"x": x[i * per : (i + 1) * per].reshape(ROWS_PER_CORE, D)}
        for i in range(N_CORES)
    ]
    nc = _get_program()
    res = run_bass_kernel_spmd(
        nc, in_maps, core_ids=list(range(N_CORES)), trace=_trace
    )
    out = np.concatenate(
        [res.results[i]["y"].reshape(per, L1, D) for i in range(N_CORES)], axis=0
    )
    if _trace:
        return out, res
    return out


# revision 5
# speedup vs baseline: 1.1460x; 1.1460x over previous
"""Top-k (k=3) row masking + renormalize, data-parallel across 8 NeuronCores.

Input  x: [128, 512, 512] f32. For each row (last axis): keep the top-3
entries (counting duplicates), zero the rest, scale kept entries by the
reciprocal of their sum.

Per-core algorithm (rows are independent; batch dim sharded 8 ways):
  - vector.max   -> top-8 values per row; entry [2] == 3rd largest == kth
  - scalar_tensor_tensor: v = (x >= kth) * x, fused row-sum accum -> s
  - vector.reciprocal: inv = 1/s  (batched over CHUNK row-blocks)
  - scalar.mul (ACT engine): out = v * inv
This is exactly the reference computation (same mask semantics incl. ties).

Layout: rows are assigned partition-major ("(p n) d"), so each partition's
blocks are contiguous in HBM -> each chunk DMA moves CHUNK*2KB contiguous
bytes per partition (large descriptors, near-peak HBM bandwidth).
"""

import sys

import numpy as np

if "/opt/trn_rl_repo" not in sys.path:
    sys.path.insert(0, "/opt/trn_rl_repo")

N_CORES = 8
B, L1, D = 128, 512, 512
ROWS_PER_CORE = (B // N_CORES) * L1  # 8192
NBLK = ROWS_PER_CORE // 128  # 64 blocks of [128, 512]
# Blocks per DMA chunk: small chunks at the ends shorten pipeline ramp
# (first compute can start after a 512KB transfer instead of 2MB) and
# drain (tail flush is recip + 2 ACT muls + a 512KB store).
CHUNKS = [2, 2, 4, 8, 8, 8, 8, 8, 8, 4, 2, 2]
assert sum(CHUNKS) == NBLK

_PROGRAM = None


def _build_program():
    from concourse import bacc, bass, tile

    mybir = bass.mybir
    f32 = mybir.dt.float32

    # Bacc (not raw Bass): its compile pass legalizes Tile's multi-wait
    # instructions, which walrus codegen rejects (one wait slot per inst).
    nc = bacc.Bacc("TRN2", target_bir_lowering=False, debug=False)
    x_in = nc.dram_tensor("x", [ROWS_PER_CORE, D], f32, kind="ExternalInput")
    y_out = nc.dram_tensor("y", [ROWS_PER_CORE, D], f32, kind="ExternalOutput")

    # [8192, 512] -> [128 partitions, 64 blocks, 512]; row (p*64+n) -> [p, n, :]
    # Partition-major: each partition's 64 blocks are contiguous in HBM.
    xv = x_in[:].rearrange("(p n) d -> p n d", n=NBLK)
    yv = y_out[:].rearrange("(p n) d -> p n d", n=NBLK)

    with tile.TileContext(nc) as tc:
        with (
            tc.tile_pool(name="xp", bufs=4) as xp,
            tc.tile_pool(name="vp", bufs=2) as vp,
            tc.tile_pool(name="op", bufs=2) as op_pool,
            tc.tile_pool(name="small", bufs=3) as sp,
        ):
            base = 0
            for ci, chunk in enumerate(CHUNKS):
                sl = slice(base, base + chunk)
                base += chunk
                # The last chunks' final muls run on Vector (idle by then);
                # everything else goes through the ACT engine.
                tail = ci >= len(CHUNKS) - 2
                xt = xp.tile([128, chunk, D], f32, tag="xt")
                vt = vp.tile([128, chunk, D], f32, tag="vt")
                ot = op_pool.tile([128, chunk, D], f32, tag="ot")
                t8 = sp.tile([128, chunk, 8], f32, tag="t8")
                st = sp.tile([128, chunk], f32, tag="st")
                iv = sp.tile([128, chunk], f32, tag="iv")

                nc.sync.dma_start(out=xt[:], in_=xv[:, sl, :])

                for j in range(chunk):
                    nc.vector.max(out=t8[:, j, :], in_=xt[:, j, :])
                    nc.vector.scalar_tensor_tensor(
                        out=vt[:, j, :],
                        in0=xt[:, j, :],
                        scalar=t8[:, j, 2:3],
                        in1=xt[:, j, :],
                        op0=mybir.AluOpType.is_ge,
                        op1=mybir.AluOpType.mult,
                        accum_out=st[:, j : j + 1],
                    )
                nc.vector.reciprocal(out=iv[:], in_=st[:])
                for j in range(chunk):
                    if tail:
                        nc.vector.tensor_scalar(
                            out=ot[:, j, :],
                            in0=vt[:, j, :],
                            scalar1=iv[:, j : j + 1],
                            scalar2=None,
                            op0=mybir.AluOpType.mult,
                        )
                    else:
                        nc.scalar.mul(
                            out=ot[:, j, :], in_=vt[:, j, :], mul=iv[:, j : j + 1]
                        )
                # Store in halves so the first half's bytes start moving
                # while the second half's muls are still running.
                if chunk >= 4:
                    h = chunk // 2
                    nc.scalar.dma_start(
                        out=yv[:, sl.start : sl.start + h, :], in_=ot[:, :h, :]
                    )
                    nc.scalar.dma_start(
                        out=yv[:, sl.start + h : sl.stop, :], in_=ot[:, h:, :]
                    )
                else:
                    nc.scalar.dma_start(out=yv[:, sl, :], in_=ot[:])

    nc.finalize()
    return nc


def _get_program():
    global _PROGRAM
    if _PROGRAM is None:
        _PROGRAM = _build_program()
    return _PROGRAM


def kernel(x: np.ndarray, _trace: bool = False):
    from concourse.bass_utils import run_bass_kernel_spmd

    x = np.ascontiguousarray(x, dtype=np.float32)
    assert x.shape == (B, L1, D), x.shape
    per = B // N_CORES
    in_maps = [
        {"x": x[i * per : (i + 1) * per].reshape(ROWS_PER_CORE, D)}
        for i in range(N_CORES)
    ]
    nc = _get_program()
    res = run_bass_kernel_spmd(
        nc, in_maps, core_ids=list(range(N_CORES)), trace=_trace
    )
    out = np.concatenate(
        [res.results[i]["y"].reshape(per, L1, D) for i in range(N_CORES)], axis=0
    )
    if _trace:
        return out, res
    return out


# revision 7
# speedup vs baseline: 1.2370x; 1.0794x over previous
"""Top-k (k=3) row masking + renormalize, data-parallel across 8 NeuronCores.

Input  x: [128, 512, 512] f32. For each row (last axis): keep the top-3
entries (counting duplicates), zero the rest, scale kept entries by the
reciprocal of their sum.

Per-core algorithm (rows are independent; batch dim sharded 8 ways):
  - vector.max   -> top-8 values per row; entry [2] == 3rd largest == kth
  - scalar_tensor_tensor: v = (x >= kth) * x, fused row-sum accum -> s
  - vector.reciprocal: inv = 1/s  (batched over CHUNK row-blocks)
  - scalar.mul (ACT engine): out = v * inv
This is exactly the reference computation (same mask semantics incl. ties).

Layout: rows are assigned partition-major ("(p n) d"), so each partition's
blocks are contiguous in HBM -> each chunk DMA moves CHUNK*2KB contiguous
bytes per partition (large descriptors, near-peak HBM bandwidth).
"""

import sys

import numpy as np

if "/opt/trn_rl_repo" not in sys.path:
    sys.path.insert(0, "/opt/trn_rl_repo")

N_CORES = 8
B, L1, D = 128, 512, 512
ROWS_PER_CORE = (B // N_CORES) * L1  # 8192
NBLK = ROWS_PER_CORE // 128  # 64 blocks of [128, 512]
# Blocks per DMA chunk: small chunks at the ends shorten pipeline ramp
# (first compute can start after a 512KB transfer instead of 2MB) and
# drain (tail flush is recip + 2 ACT muls + a 512KB store).
CHUNKS = [2, 2, 4, 8, 8, 8, 8, 8, 8, 4, 2, 2]
assert sum(CHUNKS) == NBLK

_PROGRAM = None


def _build_program():
    from concourse import bacc, bass, tile

    mybir = bass.mybir
    f32 = mybir.dt.float32

    # Bacc (not raw Bass): its compile pass legalizes Tile's multi-wait
    # instructions, which walrus codegen rejects (one wait slot per inst).
    nc = bacc.Bacc("TRN2", target_bir_lowering=False, debug=False)
    x_in = nc.dram_tensor("x", [ROWS_PER_CORE, D], f32, kind="ExternalInput")
    y_out = nc.dram_tensor("y", [ROWS_PER_CORE, D], f32, kind="ExternalOutput")

    # [8192, 512] -> [128 partitions, 64 blocks, 512]; row (p*64+n) -> [p, n, :]
    # Partition-major: each partition's 64 blocks are contiguous in HBM.
    xv = x_in[:].rearrange("(p n) d -> p n d", n=NBLK)
    yv = y_out[:].rearrange("(p n) d -> p n d", n=NBLK)

    with tile.TileContext(nc) as tc:
        with (
            tc.tile_pool(name="xp", bufs=5) as xp,
            tc.tile_pool(name="vp", bufs=3) as vp,
            tc.tile_pool(name="op", bufs=3) as op_pool,
            tc.tile_pool(name="small", bufs=4) as sp,
        ):
            base = 0
            for ci, chunk in enumerate(CHUNKS):
                sl = slice(base, base + chunk)
                base += chunk
                # The last chunks' final muls run on Vector (idle by then);
                # everything else goes through the ACT engine.
                tail = ci >= len(CHUNKS) - 2
                xt = xp.tile([128, chunk, D], f32, tag="xt")
                vt = vp.tile([128, chunk, D], f32, tag="vt")
                ot = op_pool.tile([128, chunk, D], f32, tag="ot")
                t8 = sp.tile([128, chunk, 8], f32, tag="t8")
                st = sp.tile([128, chunk], f32, tag="st")
                iv = sp.tile([128, chunk], f32, tag="iv")

                # Load in halves: the first half's blocks become readable
                # ~3us earlier, smoothing Vector's wait at chunk boundaries.
                if chunk >= 4:
                    h = chunk // 2
                    nc.sync.dma_start(
                        out=xt[:, :h, :], in_=xv[:, sl.start : sl.start + h, :]
                    )
                    nc.sync.dma_start(
                        out=xt[:, h:, :], in_=xv[:, sl.start + h : sl.stop, :]
                    )
                else:
                    nc.sync.dma_start(out=xt[:], in_=xv[:, sl, :])

                for j in range(chunk):
                    nc.vector.max(out=t8[:, j, :], in_=xt[:, j, :])
                    nc.vector.scalar_tensor_tensor(
                        out=vt[:, j, :],
                        in0=xt[:, j, :],
                        scalar=t8[:, j, 2:3],
                        in1=xt[:, j, :],
                        op0=mybir.AluOpType.is_ge,
                        op1=mybir.AluOpType.mult,
                        accum_out=st[:, j : j + 1],
                    )
                nc.vector.reciprocal(out=iv[:], in_=st[:])
                for j in range(chunk):
                    if tail:
                        nc.vector.tensor_scalar(
                            out=ot[:, j, :],
                            in0=vt[:, j, :],
                            scalar1=iv[:, j : j + 1],
                            scalar2=None,
                            op0=mybir.AluOpType.mult,
                        )
                    else:
                        nc.scalar.mul(
                            out=ot[:, j, :], in_=vt[:, j, :], mul=iv[:, j : j + 1]
                        )
                # Store in halves so the first half's bytes start moving
                # while the second half's muls are still running.
                if chunk >= 4:
                    h = chunk // 2
                    nc.scalar.dma_start(
                        out=yv[:, sl.start : sl.start + h, :], in_=ot[:, :h, :]
                    )
                    nc.scalar.dma_start(
                        out=yv[:, sl.start + h : sl.stop, :], in_=ot[:, h:, :]
                    )
                else:
                    nc.scalar.dma_start(out=yv[:, sl, :], in_=ot[:])

    nc.finalize()
    return nc


def _get_program():
    global _PROGRAM
    if _PROGRAM is None:
        _PROGRAM = _build_program()
    return _PROGRAM


def kernel(x: np.ndarray, _trace: bool = False):
    from concourse.bass_utils import run_bass_kernel_spmd

    x = np.ascontiguousarray(x, dtype=np.float32)
    assert x.shape == (B, L1, D), x.shape
    per = B // N_CORES
    in_maps = [
        {"x": x[i * per : (i + 1) * per].reshape(ROWS_PER_CORE, D)}
        for i in range(N_CORES)
    ]
    nc = _get_program()
    res = run_bass_kernel_spmd(
        nc, in_maps, core_ids=list(range(N_CORES)), trace=_trace
    )
    out = np.concatenate(
        [res.results[i]["y"].reshape(per, L1, D) for i in range(N_CORES)], axis=0
    )
    if _trace:
        return out, res
    return out


# revision 8
# speedup vs baseline: 1.2730x; 1.0291x over previous
"""Top-k (k=3) row masking + renormalize, data-parallel across 8 NeuronCores.

Input  x: [128, 512, 512] f32. For each row (last axis): keep the top-3
entries (counting duplicates), zero the rest, scale kept entries by the
reciprocal of their sum.

Per-core algorithm (rows are independent; batch dim sharded 8 ways):
  - vector.max   -> top-8 values per row; entry [2] == 3rd largest == kth
  - scalar_tensor_tensor: v = (x >= kth) * x, fused row-sum accum -> s
  - vector.reciprocal: inv = 1/s  (batched over CHUNK row-blocks)
  - scalar.mul (ACT engine): out = v * inv
This is exactly the reference computation (same mask semantics incl. ties).

Layout: rows are assigned partition-major ("(p n) d"), so each partition's
blocks are contiguous in HBM -> each chunk DMA moves CHUNK*2KB contiguous
bytes per partition (large descriptors, near-peak HBM bandwidth).
"""

import sys

import numpy as np

if "/opt/trn_rl_repo" not in sys.path:
    sys.path.insert(0, "/opt/trn_rl_repo")

N_CORES = 8
B, L1, D = 128, 512, 512
ROWS_PER_CORE = (B // N_CORES) * L1  # 8192
NBLK = ROWS_PER_CORE // 128  # 64 blocks of [128, 512]
# Blocks per DMA chunk: small chunks at the ends shorten pipeline ramp
# (first compute can start after a 512KB transfer instead of 2MB) and
# drain (tail flush is recip + 2 ACT muls + a 512KB store).
CHUNKS = [2, 2, 4, 8, 8, 8, 8, 8, 8, 4, 2, 2]
assert sum(CHUNKS) == NBLK

_PROGRAM = None


def _build_program():
    from concourse import bacc, bass, tile

    mybir = bass.mybir
    f32 = mybir.dt.float32

    # Bacc (not raw Bass): its compile pass legalizes Tile's multi-wait
    # instructions, which walrus codegen rejects (one wait slot per inst).
    nc = bacc.Bacc("TRN2", target_bir_lowering=False, debug=False)
    x_in = nc.dram_tensor("x", [ROWS_PER_CORE, D], f32, kind="ExternalInput")
    y_out = nc.dram_tensor("y", [ROWS_PER_CORE, D], f32, kind="ExternalOutput")

    # [8192, 512] -> [128 partitions, 64 blocks, 512]; row (p*64+n) -> [p, n, :]
    # Partition-major: each partition's 64 blocks are contiguous in HBM.
    xv = x_in[:].rearrange("(p n) d -> p n d", n=NBLK)
    yv = y_out[:].rearrange("(p n) d -> p n d", n=NBLK)

    with tile.TileContext(nc) as tc:
        with (
            tc.tile_pool(name="xp", bufs=7) as xp,
            tc.tile_pool(name="vp", bufs=3) as vp,
            tc.tile_pool(name="op", bufs=2) as op_pool,
            tc.tile_pool(name="small", bufs=4) as sp,
        ):
            base = 0
            for ci, chunk in enumerate(CHUNKS):
                sl = slice(base, base + chunk)
                base += chunk
                # The last chunks' final muls run on Vector (idle by then);
                # everything else goes through the ACT engine.
                tail = ci >= len(CHUNKS) - 2
                xt = xp.tile([128, chunk, D], f32, tag="xt")
                vt = vp.tile([128, chunk, D], f32, tag="vt")
                ot = op_pool.tile([128, chunk, D], f32, tag="ot")
                t8 = sp.tile([128, chunk, 8], f32, tag="t8")
                st = sp.tile([128, chunk], f32, tag="st")
                iv = sp.tile([128, chunk], f32, tag="iv")

                # Load in halves: the first half's blocks become readable
                # ~3us earlier, smoothing Vector's wait at chunk boundaries.
                if chunk >= 4:
                    h = chunk // 2
                    nc.sync.dma_start(
                        out=xt[:, :h, :], in_=xv[:, sl.start : sl.start + h, :]
                    )
                    nc.sync.dma_start(
                        out=xt[:, h:, :], in_=xv[:, sl.start + h : sl.stop, :]
                    )
                else:
                    nc.sync.dma_start(out=xt[:], in_=xv[:, sl, :])

                for j in range(chunk):
                    nc.vector.max(out=t8[:, j, :], in_=xt[:, j, :])
                    nc.vector.scalar_tensor_tensor(
                        out=vt[:, j, :],
                        in0=xt[:, j, :],
                        scalar=t8[:, j, 2:3],
                        in1=xt[:, j, :],
                        op0=mybir.AluOpType.is_ge,
                        op1=mybir.AluOpType.mult,
                        accum_out=st[:, j : j + 1],
                    )
                nc.vector.reciprocal(out=iv[:], in_=st[:])
                for j in range(chunk):
                    if tail:
                        nc.vector.tensor_scalar(
                            out=ot[:, j, :],
                            in0=vt[:, j, :],
                            scalar1=iv[:, j : j + 1],
                            scalar2=None,
                            op0=mybir.AluOpType.mult,
                        )
                    else:
                        nc.scalar.mul(
                            out=ot[:, j, :], in_=vt[:, j, :], mul=iv[:, j : j + 1]
                        )
                # Store in halves so the first half's bytes start moving
                # while the second half's muls are still running.
                if chunk >= 4:
                    h = chunk // 2
                    nc.scalar.dma_start(
                        out=yv[:, sl.start : sl.start + h, :], in_=ot[:, :h, :]
                    )
                    nc.scalar.dma_start(
                        out=yv[:, sl.start + h : sl.stop, :], in_=ot[:, h:, :]
                    )
                else:
                    nc.scalar.dma_start(out=yv[:, sl, :], in_=ot[:])

    nc.finalize()
    return nc


def _get_program():
    global _PROGRAM
    if _PROGRAM is None:
        _PROGRAM = _build_program()
    return _PROGRAM


def kernel(x: np.ndarray, _trace: bool = False):
    from concourse.bass_utils import run_bass_kernel_spmd

    x = np.ascontiguousarray(x, dtype=np.float32)
    assert x.shape == (B, L1, D), x.shape
    per = B // N_CORES
    in_maps = [
        {"x": x[i * per : (i + 1) * per].reshape(ROWS_PER_CORE, D)}
        for i in range(N_CORES)
    ]
    nc = _get_program()
    res = run_bass_kernel_spmd(
        nc, in_maps, core_ids=list(range(N_CORES)), trace=_trace
    )
    out = np.concatenate(
        [res.results[i]["y"].reshape(per, L1, D) for i in range(N_CORES)], axis=0
    )
    if _trace:
        return out, res
    return out
